# revision 9
# baseline (speedup 1.0000x reference)
"""DGCNN Trainium2 kernel: 8 graphs data-parallel over 8 NeuronCores.

Per-core pipeline (one graph, n=1920 nodes, 97-dim conv image):
  - GNN layers as dense-adjacency matmuls: P.T = z.T @ (A+I).T on PE
    (z_l = x_l @ W_l projected first, so aggregation runs at F<=32 not 128).
    deg comes from an appended ones-column (row 32 of P.T = (A+I) @ 1).
  - x_{l+1} = tanh((P + b) * (1/deg)) on DVE + ACT.
  - conv2d 13x13 as delta-packed im2col matmul: K = taps (117+65 chunks),
    M = 128 = (delta in {0,1}) x 64 channels, patches built by strided DMA
    from a zero-padded DRAM image; maxpool via DVE tensor_reduce from PSUM.

Host does only layout prep on the integer edge list (dense (A+I).T counts)
and pure transposes/reshapes; every floating-point op of the reference
(projection, aggregation, deg reciprocal, tanh, conv, bias, maxpool) runs
on device.
"""
import numpy as np
import ml_dtypes

import concourse.bacc as bacc
import concourse.mybir as mybir
import concourse.tile as tile
from concourse import bass_utils
from concourse.bass import AP
from concourse.masks import make_identity

B = 8
N = 1920
FEAT = 128
LATENT = 97
KPOOL = 30
NT = N // 128          # 15 node tiles
NW = 4                 # dst windows of 480
WIN = 480
TGROUP = 30            # conv groups = pool windows
GHB = 32               # hb per group (= 64 h rows = one pool window)
PAD_W = 109            # 97 + 12
PAD_H = N + 13         # 1933: rows 2*hb+i, hb<=959, i<=13

F32 = mybir.dt.float32
BF16 = mybir.dt.bfloat16
AX = mybir.AxisListType
ALU = mybir.AluOpType
ACTF = mybir.ActivationFunctionType

_cache = {}


def _build():
    nc = bacc.Bacc("TRN2", target_bir_lowering=False, debug=False, num_devices=B)

    nfT = nc.dram_tensor("nfT", [FEAT, N], F32, kind="ExternalInput").ap()
    AT = nc.dram_tensor("AT", [N, N], BF16, kind="ExternalInput").ap()
    Ws = [nc.dram_tensor(f"W{i}", s, F32, kind="ExternalInput").ap()
          for i, s in enumerate([[128, 32], [32, 32], [32, 32], [32, 1]])]
    bs = [nc.dram_tensor(f"b{i}", [s, 1], F32, kind="ExternalInput").ap()
          for i, s in enumerate([32, 32, 32, 1])]
    WcA = nc.dram_tensor("WcA", [117, 128], F32, kind="ExternalInput").ap()
    WcB = nc.dram_tensor("WcB", [65, 128], F32, kind="ExternalInput").ap()
    cB = nc.dram_tensor("cB", [64, 1], F32, kind="ExternalInput").ap()
    y = nc.dram_tensor("y", [64, KPOOL * LATENT], F32, kind="ExternalOutput").ap()
    imgpad = nc.dram_tensor("imgpad", [PAD_H, PAD_W], BF16, kind="Internal").ap()
    rd_dram = nc.dram_tensor("rd_dram", [1, N], F32, kind="Internal").ap()

    FOUT = [32, 32, 32, 1]

    with tile.TileContext(nc) as tc:
        with (
            tc.tile_pool(name="static", bufs=1) as st,
            tc.tile_pool(name="work", bufs=2) as wk,
        ):
            # ---- static loads ----
            at_sb = st.tile([128, NT, N], BF16, tag="at")
            nc.sync.dma_start(at_sb[:], AT.rearrange("(k p) d -> p k d", p=128))
            nfT_f = st.tile([128, N], F32, tag="nfTf")
            nc.sync.dma_start(nfT_f[:], nfT[:])
            nfT_b = st.tile([128, N], BF16, tag="nfTb")
            nc.vector.tensor_copy(nfT_b[:], nfT_f[:])
            w_sb = []
            for i, s in enumerate([[128, 32], [32, 32], [32, 32], [32, 1]]):
                wf = wk.tile(s, F32, tag=f"wf{i}")
                nc.sync.dma_start(wf[:], Ws[i][:])
                wb = st.tile(s, BF16, tag=f"wb{i}")
                nc.vector.tensor_copy(wb[:], wf[:])
                w_sb.append(wb)
            b_sb = []
            for i, s in enumerate([32, 32, 32, 1]):
                bb = st.tile([s, 1], F32, tag=f"bb{i}")
                nc.sync.dma_start(bb[:], bs[i][:])
                b_sb.append(bb)
            wcA_f = wk.tile([117, 128], F32, tag="wcAf")
            nc.sync.dma_start(wcA_f[:], WcA[:])
            wcA = st.tile([117, 128], BF16, tag="wcA")
            nc.vector.tensor_copy(wcA[:], wcA_f[:])
            wcB_f = wk.tile([65, 128], F32, tag="wcBf")
            nc.sync.dma_start(wcB_f[:], WcB[:])
            wcB = st.tile([65, 128], BF16, tag="wcB")
            nc.vector.tensor_copy(wcB[:], wcB_f[:])
            cB_sb = st.tile([64, 1], F32, tag="cB")
            nc.sync.dma_start(cB_sb[:], cB[:])

            # imgT rows: 0..31 x1, 32..63 x2, 64..95 x3, 96 x4, rest zero
            imgT = st.tile([128, N], BF16, tag="imgT")
            nc.gpsimd.memset(imgT[:], 0.0)
            rd = st.tile([1, N], F32, tag="rd")
            rd32 = st.tile([32, N], F32, tag="rd32")
            tmp = st.tile([32, N], F32, tag="tmp")
            xts = [st.tile([32, N], BF16, tag=f"xt{i}", name=f"xt{i}") for i in range(4)]

            # ---- GNN layers ----
            with tc.tile_pool(name="psg", bufs=2, space="PSUM") as psg:
                for l in range(4):
                    fo = FOUT[l]
                    z = wk.tile([128, NT, 33], BF16, tag="z")
                    nc.gpsimd.memset(z[:], 0.0)
                    if l == 0:
                        nc.gpsimd.memset(z[:, :, 32], 1.0)
                    for nt in range(NT):
                        zps = psg.tile([128, 512], F32, tag="zps")
                        if l == 0:
                            lhsT = nfT_b[:, nt * 128:(nt + 1) * 128]
                        else:
                            lhsT = xts[l - 1][:, nt * 128:(nt + 1) * 128]
                        nc.tensor.matmul(zps[:, :fo], lhsT, w_sb[l][:],
                                         start=True, stop=True)
                        nc.vector.tensor_copy(z[:, nt, :fo], zps[:, :fo])
                    # aggregation: P.T[33, N] = z.T @ (A+I).T
                    pps = psg.tile([33, NW, 512], F32, tag="pps", bufs=1)
                    for w in range(NW):
                        for k in range(NT):
                            nc.tensor.matmul(
                                pps[:, w, :WIN], z[:, k, :],
                                at_sb[:, k, w * WIN:(w + 1) * WIN],
                                start=(k == 0), stop=(k == NT - 1))
                    if l == 0:
                        for w in range(NW):
                            nc.vector.reciprocal(
                                rd[:, w * WIN:(w + 1) * WIN], pps[32:33, w, :WIN])
                        nc.sync.dma_start(rd_dram[:], rd[:])
                        nc.sync.dma_start(
                            rd32[:], AP(rd_dram.tensor, 0, [[0, 32], [1, N]]))
                    # x_{l+1} = tanh((P + b) * rd)
                    out_base = 96 if l == 3 else 32 * l
                    for w in range(NW):
                        sl = slice(w * WIN, (w + 1) * WIN)
                        nc.vector.tensor_scalar_add(
                            tmp[:fo, sl], pps[:fo, w, :WIN], b_sb[l][:])
                        nc.vector.tensor_tensor(
                            out=tmp[:fo, sl], in0=tmp[:fo, sl],
                            in1=rd32[:fo, sl], op=ALU.mult)
                    nc.scalar.activation(
                        xts[l][:fo, :], tmp[:fo, :], ACTF.Tanh)
                    nc.vector.tensor_copy(
                        imgT[out_base:out_base + fo, :], xts[l][:fo, :])

                # ---- transpose to image rows ----
                ident = st.tile([128, 128], BF16, tag="ident")
                make_identity(nc, ident[:])
                imgrows = st.tile([128, NT, LATENT], BF16, tag="imgrows")
                for t in range(NT):
                    tps = psg.tile([128, 512], BF16, tag="tps")
                    nc.tensor.transpose(tps[:, :128],
                                        imgT[:, t * 128:(t + 1) * 128], ident[:])
                    nc.vector.tensor_copy(imgrows[:, t, :], tps[:, :LATENT])

            # ---- padded image in DRAM ----
            zr = st.tile([128, 16 * PAD_W], BF16, tag="zr")
            nc.gpsimd.memset(zr[:], 0.0)
            nc.sync.dma_start(
                imgpad[:1920, :].rearrange("(k p) d -> p k d", p=128),
                zr[:, :15 * PAD_W].rearrange("p (k d) -> p k d", d=PAD_W))
            nc.sync.dma_start(imgpad[1920:, :], zr[:13, :PAD_W])
            nc.sync.dma_start(
                imgpad[6:1926, 6:103].rearrange("(k p) d -> p k d", p=128),
                imgrows[:])

            # ---- conv + maxpool ----
            out_sb = st.tile([128, KPOOL * LATENT], F32, tag="osb")
            with (
                tc.tile_pool(name="patch", bufs=2) as ppool,
                tc.tile_pool(name="psc", bufs=2, space="PSUM") as psc,
            ):
                for g in range(TGROUP):
                    sA = ppool.tile([117, GHB, LATENT], BF16, tag="sA")
                    sB = ppool.tile([65, GHB, LATENT], BF16, tag="sB")
                    for i in range(9):
                        nc.sync.dma_start(
                            sA[i * 13:(i + 1) * 13, :, :],
                            AP(imgpad.tensor, (64 * g + i) * PAD_W,
                               [[1, 13], [2 * PAD_W, GHB], [1, LATENT]]))
                    for i in range(5):
                        nc.sync.dma_start(
                            sB[i * 13:(i + 1) * 13, :, :],
                            AP(imgpad.tensor, (64 * g + 9 + i) * PAD_W,
                               [[1, 13], [2 * PAD_W, GHB], [1, LATENT]]))
                    waccs = []
                    for half in range(2):
                        cps = psc.tile([128, 4, 512], F32, tag="cps")
                        for t in range(4):
                            tt = 4 * half + t
                            nc.tensor.matmul(
                                cps[:, t, :388], wcA[:],
                                sA[:, 4 * tt:4 * tt + 4, :],
                                start=True, stop=False)
                        for t in range(4):
                            tt = 4 * half + t
                            nc.tensor.matmul(
                                cps[:, t, :388], wcB[:],
                                sB[:, 4 * tt:4 * tt + 4, :],
                                start=False, stop=True)
                        wacc = wk.tile([128, LATENT], F32, tag="wacc")
                        cap = cps[:]
                        rin = AP(cap.tensor, cap.offset,
                                 [cap.ap[0], [1, LATENT], [512, 4], [LATENT, 4]])
                        nc.vector.tensor_reduce(
                            out=wacc[:], in_=rin, axis=AX.XY, op=ALU.max)
                        waccs.append(wacc)
                    nc.vector.tensor_tensor(
                        out=out_sb[:, g * LATENT:(g + 1) * LATENT],
                        in0=waccs[0][:], in1=waccs[1][:], op=ALU.max)
            shift = st.tile([64, KPOOL * LATENT], F32, tag="shift")
            nc.sync.dma_start(shift[:], out_sb[64:128, :])
            nc.vector.tensor_tensor(
                out=out_sb[:64, :], in0=out_sb[:64, :], in1=shift[:], op=ALU.max)
            nc.vector.tensor_scalar_add(out_sb[:64, :], out_sb[:64, :], cB_sb[:])
            nc.sync.dma_start(y[:], out_sb[:64, :])

    nc.compile()
    return nc


def _host_prep(nodeFeats, src, dst, W0, b0, W1, b1, W2, b2, W3, b3, convW, convB):
    convW = np.asarray(convW, np.float32)
    wcA = np.zeros((117, 128), np.float32)
    wcB = np.zeros((65, 128), np.float32)
    for i in range(14):
        for j in range(13):
            for d in range(2):
                a = i - d
                if 0 <= a <= 12:
                    col = slice(d * 64, d * 64 + 64)
                    if i <= 8:
                        wcA[i * 13 + j, col] = convW[:, 0, a, j]
                    else:
                        wcB[(i - 9) * 13 + j, col] = convW[:, 0, a, j]
    nodeFeats = np.asarray(nodeFeats, np.float32)
    src = np.asarray(src).reshape(B, -1)
    dst = np.asarray(dst).reshape(B, -1)
    in_maps = []
    for g in range(B):
        nf = nodeFeats[g * N:(g + 1) * N]
        s = src[g].astype(np.int64) - g * N
        d = dst[g].astype(np.int64) - g * N
        at = np.zeros((N, N), np.float32)
        np.add.at(at, (s, d), 1.0)
        at[np.arange(N), np.arange(N)] += 1.0
        in_maps.append({
            "nfT": np.ascontiguousarray(nf.T),
            "AT": at.astype(ml_dtypes.bfloat16),
            "W0": np.asarray(W0, np.float32), "W1": np.asarray(W1, np.float32),
            "W2": np.asarray(W2, np.float32), "W3": np.asarray(W3, np.float32),
            "b0": np.asarray(b0, np.float32).reshape(32, 1),
            "b1": np.asarray(b1, np.float32).reshape(32, 1),
            "b2": np.asarray(b2, np.float32).reshape(32, 1),
            "b3": np.asarray(b3, np.float32).reshape(1, 1),
            "WcA": wcA, "WcB": wcB,
            "cB": np.asarray(convB, np.float32).reshape(64, 1),
        })
    return in_maps


def kernel(**inputs) -> np.ndarray:
    if "nc" not in _cache:
        _cache["nc"] = _build()
    nc = _cache["nc"]
    in_maps = _host_prep(**inputs)
    res = bass_utils.run_bass_kernel_spmd(nc, in_maps, core_ids=list(range(B)))
    out = np.stack([np.asarray(res.results[g]["y"], np.float32)
                    .reshape(64, KPOOL, LATENT) for g in range(B)])
    return out


# revision 11
# speedup vs baseline: 1.1362x; 1.1362x over previous
"""DGCNN Trainium2 kernel: 8 graphs data-parallel over 8 NeuronCores.

Per-core pipeline (one graph, n=1920 nodes, 97-dim conv image):
  - GNN layers as dense-adjacency matmuls: P.T = z.T @ (A+I).T on PE
    (z_l = x_l @ W_l projected first, so aggregation runs at F<=32 not 128).
    deg comes from an appended ones-column (row 32 of P.T = (A+I) @ 1).
  - x_{l+1} = tanh((P + b) * (1/deg)) on DVE + ACT.
  - conv2d 13x13 as delta-packed im2col matmul: K = taps (117+65 chunks),
    M = 128 = (delta in {0,1}) x 64 channels, patches built by strided DMA
    from a zero-padded DRAM image; maxpool via DVE tensor_reduce from PSUM.

Host does only layout prep on the integer edge list (dense (A+I).T counts)
and pure transposes/reshapes; every floating-point op of the reference
(projection, aggregation, deg reciprocal, tanh, conv, bias, maxpool) runs
on device.
"""
import numpy as np
import ml_dtypes

import concourse.bacc as bacc
import concourse.mybir as mybir
import concourse.tile as tile
from concourse import bass_utils
from concourse.bass import AP
from concourse.masks import make_identity

B = 8
N = 1920
FEAT = 128
LATENT = 97
KPOOL = 30
NT = N // 128          # 15 node tiles
NW = 4                 # dst windows of 480
WIN = 480
TGROUP = 30            # conv groups = pool windows
GHB = 32               # hb per group (= 64 h rows = one pool window)
PAD_W = 109            # 97 + 12
PAD_H = N + 13         # 1933: rows 2*hb+i, hb<=959, i<=13

F32 = mybir.dt.float32
BF16 = mybir.dt.bfloat16
AX = mybir.AxisListType
ALU = mybir.AluOpType
ACTF = mybir.ActivationFunctionType

_cache = {}


def _build():
    nc = bacc.Bacc("TRN2", target_bir_lowering=False, debug=False, num_devices=B)

    nfT = nc.dram_tensor("nfT", [FEAT, N], F32, kind="ExternalInput").ap()
    AT = nc.dram_tensor("AT", [N, N], BF16, kind="ExternalInput").ap()
    Ws = [nc.dram_tensor(f"W{i}", s, F32, kind="ExternalInput").ap()
          for i, s in enumerate([[128, 32], [32, 32], [32, 32], [32, 1]])]
    bs = [nc.dram_tensor(f"b{i}", [s, 1], F32, kind="ExternalInput").ap()
          for i, s in enumerate([32, 32, 32, 1])]
    WcA = nc.dram_tensor("WcA", [117, 128], F32, kind="ExternalInput").ap()
    WcB = nc.dram_tensor("WcB", [65, 128], F32, kind="ExternalInput").ap()
    cB = nc.dram_tensor("cB", [64, 1], F32, kind="ExternalInput").ap()
    y = nc.dram_tensor("y", [64, KPOOL * LATENT], F32, kind="ExternalOutput").ap()
    imgpad = nc.dram_tensor("imgpad", [PAD_H, PAD_W], BF16, kind="Internal").ap()
    rd_dram = nc.dram_tensor("rd_dram", [1, N], F32, kind="Internal").ap()

    FOUT = [32, 32, 32, 1]

    with tile.TileContext(nc) as tc:
        with (
            tc.tile_pool(name="static", bufs=1) as st,
            tc.tile_pool(name="work", bufs=2) as wk,
        ):
            # ---- static loads ----
            at_sb = st.tile([128, NT, N], BF16, tag="at")
            at_re = AT.rearrange("(k p) d -> p k d", p=128)
            for w in range(NW):
                nc.sync.dma_start(at_sb[:, :, w * WIN:(w + 1) * WIN],
                                  at_re[:, :, w * WIN:(w + 1) * WIN])
            nfT_f = st.tile([128, N], F32, tag="nfTf")
            nc.sync.dma_start(nfT_f[:], nfT[:])
            nfT_b = st.tile([128, N], BF16, tag="nfTb")
            nc.vector.tensor_copy(nfT_b[:], nfT_f[:])
            w_sb = []
            for i, s in enumerate([[128, 32], [32, 32], [32, 32], [32, 1]]):
                wf = wk.tile(s, F32, tag=f"wf{i}")
                nc.sync.dma_start(wf[:], Ws[i][:])
                wb = st.tile(s, BF16, tag=f"wb{i}")
                nc.vector.tensor_copy(wb[:], wf[:])
                w_sb.append(wb)
            b_sb = []
            for i, s in enumerate([32, 32, 32, 1]):
                bb = st.tile([s, 1], F32, tag=f"bb{i}")
                nc.sync.dma_start(bb[:], bs[i][:])
                b_sb.append(bb)
            wcA_f = wk.tile([117, 128], F32, tag="wcAf")
            nc.sync.dma_start(wcA_f[:], WcA[:])
            wcA = st.tile([117, 128], BF16, tag="wcA")
            nc.vector.tensor_copy(wcA[:], wcA_f[:])
            wcB_f = wk.tile([65, 128], F32, tag="wcBf")
            nc.sync.dma_start(wcB_f[:], WcB[:])
            wcB = st.tile([65, 128], BF16, tag="wcB")
            nc.vector.tensor_copy(wcB[:], wcB_f[:])
            cB_sb = st.tile([64, 1], F32, tag="cB")
            nc.sync.dma_start(cB_sb[:], cB[:])

            # imgT rows: 0..31 x1, 32..63 x2, 64..95 x3, 96 x4, rest zero
            imgT = st.tile([128, N], BF16, tag="imgT")
            nc.gpsimd.memset(imgT[:], 0.0)
            rd = st.tile([1, N], F32, tag="rd")
            rd32 = st.tile([32, N], F32, tag="rd32")
            tmp = st.tile([32, N], F32, tag="tmp")
            xts = [st.tile([32, N], BF16, tag=f"xt{i}", name=f"xt{i}") for i in range(4)]

            # ---- GNN layers ----
            with tc.tile_pool(name="psg", bufs=2, space="PSUM") as psg:
                for l in range(4):
                    fo = FOUT[l]
                    z = wk.tile([128, NT, 33], BF16, tag="z")
                    nc.gpsimd.memset(z[:], 0.0)
                    if l == 0:
                        nc.gpsimd.memset(z[:, :, 32], 1.0)
                    for nt in range(NT):
                        zps = psg.tile([128, 512], F32, tag="zps")
                        if l == 0:
                            lhsT = nfT_b[:, nt * 128:(nt + 1) * 128]
                        else:
                            lhsT = xts[l - 1][:, nt * 128:(nt + 1) * 128]
                        nc.tensor.matmul(zps[:, :fo], lhsT, w_sb[l][:],
                                         start=True, stop=True)
                        nc.vector.tensor_copy(z[:, nt, :fo], zps[:, :fo])
                    # aggregation: P.T[33, N] = z.T @ (A+I).T
                    ppsw = [psg.tile([33, 512], F32, tag=f"pps{w}",
                                     name=f"pps_l{l}w{w}", bufs=1)
                            for w in range(NW)]
                    for w in range(NW):
                        for k in range(NT):
                            nc.tensor.matmul(
                                ppsw[w][:, :WIN], z[:, k, :],
                                at_sb[:, k, w * WIN:(w + 1) * WIN],
                                start=(k == 0), stop=(k == NT - 1))
                    if l == 0:
                        for w in range(NW):
                            nc.vector.reciprocal(
                                rd[:, w * WIN:(w + 1) * WIN], ppsw[w][32:33, :WIN])
                        nc.sync.dma_start(rd_dram[:], rd[:])
                        nc.sync.dma_start(
                            rd32[:], AP(rd_dram.tensor, 0, [[0, 32], [1, N]]))
                    # x_{l+1} = tanh((P + b) * rd)
                    out_base = 96 if l == 3 else 32 * l
                    for w in range(NW):
                        sl = slice(w * WIN, (w + 1) * WIN)
                        nc.vector.tensor_scalar_add(
                            tmp[:fo, sl], ppsw[w][:fo, :WIN], b_sb[l][:])
                        nc.vector.tensor_tensor(
                            out=tmp[:fo, sl], in0=tmp[:fo, sl],
                            in1=rd32[:fo, sl], op=ALU.mult)
                    for w in range(NW):
                        sl = slice(w * WIN, (w + 1) * WIN)
                        nc.scalar.activation(
                            xts[l][:fo, sl], tmp[:fo, sl], ACTF.Tanh)
                    nc.vector.tensor_copy(
                        imgT[out_base:out_base + fo, :], xts[l][:fo, :])

                # ---- transpose to image rows ----
                ident = st.tile([128, 128], BF16, tag="ident")
                make_identity(nc, ident[:])
                imgrows = st.tile([128, NT, LATENT], BF16, tag="imgrows")
                for t in range(NT):
                    tps = psg.tile([128, 512], BF16, tag="tps")
                    nc.tensor.transpose(tps[:, :128],
                                        imgT[:, t * 128:(t + 1) * 128], ident[:])
                    nc.vector.tensor_copy(imgrows[:, t, :], tps[:, :LATENT])

            # ---- padded image in DRAM ----
            zr = st.tile([128, 16 * PAD_W], BF16, tag="zr")
            nc.gpsimd.memset(zr[:], 0.0)
            nc.sync.dma_start(
                imgpad[:1920, :].rearrange("(k p) d -> p k d", p=128),
                zr[:, :15 * PAD_W].rearrange("p (k d) -> p k d", d=PAD_W))
            nc.sync.dma_start(imgpad[1920:, :], zr[:13, :PAD_W])
            nc.sync.dma_start(
                imgpad[6:1926, 6:103].rearrange("(k p) d -> p k d", p=128),
                imgrows[:])

            # ---- conv + maxpool ----
            out_sb = st.tile([128, KPOOL * LATENT], F32, tag="osb")
            with (
                tc.tile_pool(name="patch", bufs=2) as ppool,
                tc.tile_pool(name="psc", bufs=2, space="PSUM") as psc,
            ):
                for g in range(TGROUP):
                    sA = ppool.tile([117, GHB, LATENT], BF16, tag="sA")
                    sB = ppool.tile([65, GHB, LATENT], BF16, tag="sB")
                    for i in range(9):
                        nc.sync.dma_start(
                            sA[i * 13:(i + 1) * 13, :, :],
                            AP(imgpad.tensor, (64 * g + i) * PAD_W,
                               [[1, 13], [2 * PAD_W, GHB], [1, LATENT]]))
                    for i in range(5):
                        nc.sync.dma_start(
                            sB[i * 13:(i + 1) * 13, :, :],
                            AP(imgpad.tensor, (64 * g + 9 + i) * PAD_W,
                               [[1, 13], [2 * PAD_W, GHB], [1, LATENT]]))
                    waccs = []
                    for half in range(2):
                        cps = psc.tile([128, 4, 512], F32, tag="cps")
                        for t in range(4):
                            tt = 4 * half + t
                            nc.tensor.matmul(
                                cps[:, t, :388], wcA[:],
                                sA[:, 4 * tt:4 * tt + 4, :],
                                start=True, stop=False)
                        for t in range(4):
                            tt = 4 * half + t
                            nc.tensor.matmul(
                                cps[:, t, :388], wcB[:],
                                sB[:, 4 * tt:4 * tt + 4, :],
                                start=False, stop=True)
                        wacc = wk.tile([128, LATENT], F32, tag="wacc")
                        cap = cps[:]
                        rin = AP(cap.tensor, cap.offset,
                                 [cap.ap[0], [1, LATENT], [512, 4], [LATENT, 4]])
                        nc.vector.tensor_reduce(
                            out=wacc[:], in_=rin, axis=AX.XY, op=ALU.max)
                        waccs.append(wacc)
                    nc.vector.tensor_tensor(
                        out=out_sb[:, g * LATENT:(g + 1) * LATENT],
                        in0=waccs[0][:], in1=waccs[1][:], op=ALU.max)
            shift = st.tile([64, KPOOL * LATENT], F32, tag="shift")
            nc.sync.dma_start(shift[:], out_sb[64:128, :])
            nc.vector.tensor_tensor(
                out=out_sb[:64, :], in0=out_sb[:64, :], in1=shift[:], op=ALU.max)
            nc.vector.tensor_scalar_add(out_sb[:64, :], out_sb[:64, :], cB_sb[:])
            nc.sync.dma_start(y[:], out_sb[:64, :])

    nc.compile()
    return nc


def _host_prep(nodeFeats, src, dst, W0, b0, W1, b1, W2, b2, W3, b3, convW, convB):
    convW = np.asarray(convW, np.float32)
    wcA = np.zeros((117, 128), np.float32)
    wcB = np.zeros((65, 128), np.float32)
    for i in range(14):
        for j in range(13):
            for d in range(2):
                a = i - d
                if 0 <= a <= 12:
                    col = slice(d * 64, d * 64 + 64)
                    if i <= 8:
                        wcA[i * 13 + j, col] = convW[:, 0, a, j]
                    else:
                        wcB[(i - 9) * 13 + j, col] = convW[:, 0, a, j]
    nodeFeats = np.asarray(nodeFeats, np.float32)
    src = np.asarray(src).reshape(B, -1)
    dst = np.asarray(dst).reshape(B, -1)
    in_maps = []
    for g in range(B):
        nf = nodeFeats[g * N:(g + 1) * N]
        s = src[g].astype(np.int64) - g * N
        d = dst[g].astype(np.int64) - g * N
        at = np.zeros((N, N), np.float32)
        np.add.at(at, (s, d), 1.0)
        at[np.arange(N), np.arange(N)] += 1.0
        in_maps.append({
            "nfT": np.ascontiguousarray(nf.T),
            "AT": at.astype(ml_dtypes.bfloat16),
            "W0": np.asarray(W0, np.float32), "W1": np.asarray(W1, np.float32),
            "W2": np.asarray(W2, np.float32), "W3": np.asarray(W3, np.float32),
            "b0": np.asarray(b0, np.float32).reshape(32, 1),
            "b1": np.asarray(b1, np.float32).reshape(32, 1),
            "b2": np.asarray(b2, np.float32).reshape(32, 1),
            "b3": np.asarray(b3, np.float32).reshape(1, 1),
            "WcA": wcA, "WcB": wcB,
            "cB": np.asarray(convB, np.float32).reshape(64, 1),
        })
    return in_maps


def kernel(**inputs) -> np.ndarray:
    if "nc" not in _cache:
        _cache["nc"] = _build()
    nc = _cache["nc"]
    in_maps = _host_prep(**inputs)
    res = bass_utils.run_bass_kernel_spmd(nc, in_maps, core_ids=list(range(B)))
    out = np.stack([np.asarray(res.results[g]["y"], np.float32)
                    .reshape(64, KPOOL, LATENT) for g in range(B)])
    return out
